# revision 17
# baseline (speedup 1.0000x reference)
"""KMaxPool1d (top-k=8 along last dim, positional order) on 8 trn2 NeuronCores.

Contract: kernel(**inputs) takes the FULL inputs
    inputs: [32, 512, 4096] float32
    top_k:  scalar (== 8)
and returns the FULL output [32, 512, 8] float32, equal to
    jnp.take_along_axis(inputs, jnp.sort(jax.lax.top_k(inputs, 8)[1], -1), -1)

Strategy: pure data parallel over rows. The (32, 512) leading dims flatten to
16384 independent rows of 4096; each of the 8 cores gets a contiguous slab of
2048 rows = 16 tiles of [128 partitions x 4096].

The DVE is the bottleneck engine (the two unavoidable full scans cost
~8.85us/tile vs ~5.9us/tile of DMA), so the per-tile loop puts NOTHING else
on the DVE:
  max        -> top-8 values, descending                    (full scan)
  max_index  -> their positions; duplicate values match
                successive occurrences, which reproduces
                jax.lax.top_k's lowest-index-first tie-break (full scan)
The position-ordering runs ONCE, batched over all 16 tiles, at the end
(pairwise ranks instead of a per-tile 8-wide sort; indices are distinct):
  rank[r] = #{s: idx[s] < idx[r]}
  out[j]  = sum_r vals[r] * (rank[r] == j)
"""

import sys

if "/opt/trn_rl_repo" not in sys.path:
    sys.path.insert(0, "/opt/trn_rl_repo")

import numpy as np

B, C, L, K = 32, 512, 4096, 8
N_CORES = 8
ROWS = B * C
ROWS_PER_CORE = ROWS // N_CORES  # 2048

_NC_CACHE = {}


def install_ntff_hook():
    """Register the axon NTFF profiling hook if the container lost it.

    trn_boot.py tries to register this hook at interpreter start but
    degrades silently when the image's antenv package lacks the tiny
    axon_hooks get/set module. Recreate the module in sys.modules and
    register the ctypes hook so run_bass_kernel_spmd(trace=True) can
    capture real NTFF profiles (true HW exec time) instead of falling
    back to wall clock. Best-effort: returns False when unavailable.
    """
    import types

    try:
        import antenv  # noqa: F401

        if "antenv.axon_hooks" not in sys.modules:
            mod = types.ModuleType("antenv.axon_hooks")
            mod._HOOK = None

            def set_axon_ntff_profile_hook(h):
                mod._HOOK = h

            def get_axon_ntff_profile_hook():
                return mod._HOOK

            mod.set_axon_ntff_profile_hook = set_axon_ntff_profile_hook
            mod.get_axon_ntff_profile_hook = get_axon_ntff_profile_hook
            sys.modules["antenv.axon_hooks"] = mod

        hooks = sys.modules["antenv.axon_hooks"]
        if hooks.get_axon_ntff_profile_hook() is None:
            from trn_agent_boot.trn_boot import _ntff_profile_via_ctypes

            hook = _ntff_profile_via_ctypes("/opt/axon/libaxon_pjrt.so")
            if hook is None:
                return False
            hooks.set_axon_ntff_profile_hook(hook)
        return True
    except Exception:
        return False


def _build_nc(rows_per_core=ROWS_PER_CORE):
    import concourse.bass as bass
    import concourse.bacc as bacc
    import concourse.mybir as mybir
    from concourse.tile import TileContext

    F32 = mybir.dt.float32
    U32 = mybir.dt.uint32

    # Bacc (not plain Bass): its compile() pass splits multi-sem waits into
    # event-semaphore nops — walrus rejects >1 sync wait per instruction.
    nc = bacc.Bacc(None)
    x = nc.dram_tensor("x", [rows_per_core, L], F32, kind="ExternalInput")
    y = nc.dram_tensor("y", [rows_per_core, K], F32, kind="ExternalOutput")
    ntiles = rows_per_core // 128

    with TileContext(nc) as tc:
        with (
            # bufs=8 with exactly one DMA per tile keeps slot reuse on the
            # same SWDGE queue (Tile round-robins 8 queues), so each load
            # needs at most one semaphore wait — the DIRECT2D DMA struct
            # can't encode more.
            tc.tile_pool(name="xp", bufs=4) as xp,
            tc.tile_pool(name="op", bufs=1) as op,
        ):
            # Dedicated (never-reused) buffers: the DVE's only wait per
            # tile is its input DMA.
            U16 = mybir.dt.uint16
            vall = op.tile([128, ntiles, K], F32)
            iall = op.tile([128, ntiles, K], U16)
            ltb = op.tile([128, ntiles, K, K], U16)
            eqb = op.tile([128, ntiles, K, K], U16)
            prb = op.tile([128, ntiles, K, K], F32)
            rnk = op.tile([128, ntiles, K], U16)
            out_all = op.tile([128, ntiles, K], F32)
            jota = op.tile([128, K], U16)

            vh = op.tile([128, 2, K], F32)
            xsrc = x.rearrange("(p t) l -> t p l", t=ntiles)
            for t in range(ntiles):
                xt = xp.tile([128, L], F32, tag="xt")
                # Row-interleaved mapping: tile t / partition p holds row
                # p*ntiles + t, so the final store is 512B-contiguous per
                # partition instead of 2048 scattered 32B chunks.
                # HWDGE (SP engine): hardware descriptor generation.
                if t == 0:
                    # Split the first load in half so the first scan can
                    # start ~3us earlier (nothing prefetches tile 0).
                    # max8 over the concatenated per-half top-8s is exact,
                    # incl. ties: h1's values precede h2's, matching the
                    # lowest-index-first tie-break.
                    half = L // 2
                    nc.sync.dma_start(xt[:, :half], xsrc[t][:, :half])
                    nc.sync.dma_start(xt[:, half:], xsrc[t][:, half:])
                    nc.vector.max(vh[:, 0, :], xt[:, :half])
                    nc.vector.max(vh[:, 1, :], xt[:, half:])
                    nc.vector.max(vall[:, t, :], vh[:].rearrange("p a k -> p (a k)"))
                else:
                    nc.sync.dma_start(xt[:], xsrc[t])
                    # The two unavoidable full scans — the only DVE work.
                    nc.vector.max(vall[:, t, :], xt[:])
                nc.vector.max_index(iall[:, t, :], vall[:, t, :], xt[:])

            # Batched position-ordering over all tiles.
            # 0..7 fits exactly in f32.
            nc.gpsimd.iota(
                jota[:], pattern=[[1, K]], base=0, channel_multiplier=0
            )

            sh = [128, ntiles, K, K]
            a_r = iall[:].rearrange("p t (r o) -> p t r o", o=1).to_broadcast(sh)
            a_s = iall[:].rearrange("p t (o s) -> p t o s", o=1).to_broadcast(sh)
            lt = ltb[:]
            # u16 comparisons get the 2-byte 2x DVE mode.
            nc.vector.tensor_tensor(lt, a_s, a_r, op=mybir.AluOpType.is_lt)
            rank = rnk[:]
            # rank sums 8 one-bits — exact in u16.
            with nc.allow_low_precision(reason="u16 sum of 8 booleans is exact"):
                nc.vector.tensor_reduce(
                    rank, lt, axis=mybir.AxisListType.X, op=mybir.AluOpType.add
                )

            # eq[t,j,r] = (rank[t,r] == j); out[t,j] = sum_r eq * vals[t,r]
            eq = eqb[:]
            r_b = rank.rearrange("p t (o r) -> p t o r", o=1).to_broadcast(sh)
            j_b = (
                jota[:]
                .rearrange("p (o j oo) -> p o j oo", o=1, oo=1)
                .to_broadcast(sh)
            )
            nc.vector.tensor_tensor(eq, r_b, j_b, op=mybir.AluOpType.is_equal)
            v_b = vall[:].rearrange("p t (o r) -> p t o r", o=1).to_broadcast(sh)
            pr = prb[:]
            # Halved mult+reduce+store: the first half's store DMA overlaps
            # the second half's compute, and the final store is 32KB not
            # 64KB. y is 512B-contiguous per partition (interleaved map).
            h = ntiles // 2
            yv = y.rearrange("(p t) k -> p t k", t=ntiles)
            mult, addop, X = (
                mybir.AluOpType.mult,
                mybir.AluOpType.add,
                mybir.AxisListType.X,
            )
            nc.vector.tensor_tensor(pr[:, :h], eq[:, :h], v_b[:, :h], op=mult)
            nc.vector.tensor_reduce(out_all[:, :h], pr[:, :h], axis=X, op=addop)
            nc.scalar.dma_start(yv[:, :h], out_all[:, :h])
            nc.vector.tensor_tensor(pr[:, h:], eq[:, h:], v_b[:, h:], op=mult)
            nc.vector.tensor_reduce(out_all[:, h:], pr[:, h:], axis=X, op=addop)
            nc.scalar.dma_start(yv[:, h:], out_all[:, h:])
    nc.finalize()  # runs Bacc.compile(): reg alloc + sync-wait splitting
    return nc


def _get_nc():
    if "nc" not in _NC_CACHE:
        _NC_CACHE["nc"] = _build_nc()
    return _NC_CACHE["nc"]


def run_spmd(flat_x, trace=False):
    """flat_x: [16384, 4096] f32. Returns ([16384, 8] f32, exec_time_ns|None)."""
    from concourse.bass_utils import run_bass_kernel_spmd

    if trace:
        install_ntff_hook()
    nc = _get_nc()
    shards = np.split(np.ascontiguousarray(flat_x), N_CORES, axis=0)
    res = run_bass_kernel_spmd(
        nc,
        [{"x": s} for s in shards],
        list(range(N_CORES)),
        trace=trace,
        trace_cores=list(range(N_CORES)) if trace else None,
    )
    out = np.concatenate([res.results[c]["y"] for c in range(N_CORES)], axis=0)
    return out, res.exec_time_ns


def kernel(inputs, top_k):
    assert int(top_k) == K, f"kernel hardcodes top_k={K}, got {top_k}"
    x = np.asarray(inputs, dtype=np.float32).reshape(ROWS, L)
    out, _ = run_spmd(x)
    return out.reshape(B, C, K)
